# revision 1
# baseline (speedup 1.0000x reference)
"""Trainium2 Bass kernel for nn_HSR_2_25116968747549 (gnn_message_passing).

The reference's edge construction (`tile(B,1).reshape(2,-1)`, the preserved
index-mixing bug) makes `edge_src == edge_dst` for every edge: all edges are
self-edges.  For a segment whose edges all share src == dst == n,
    out[n] = sum_e alpha_e * xl[src_e] = xl[n] * sum_e alpha_e = xl[n]
regardless of the attention logits, so each GATv2 layer collapses to the dense
affine map  x -> (x @ Wl + bl + cb) @ linw  and Wr/br/att never affect the
output.  The whole network is then

    t   = leaky_relu(x @ M1 + v1, 0.01)          M1 = Wl1@linw1@w1  (64x64)
    t_n = layernorm(t) * gamma + beta
    out = leaky_relu(t_n @ M2 + v2, 0.01)        M2 folded likewise

LayerNorm is folded further: (t - mu) = t @ C with C = I - J/64, and the
per-row rstd scale commutes past the second matmul, so on device we compute

    t    = leaky_relu(x @ M1 + v1)               (M1,v1 folded on host)
    a_r  = rsqrt(mean(t^2) - mean(t)^2 + eps)    (per row)
    out  = leaky_relu((a_r * t) @ M2c + v2)      M2c = C @ diag(gamma) @ M2

Sharding: data-parallel over windows; core c owns rows [1024c, 1024(c+1)).
Host passes x transposed+augmented ([65, 1024] feature-major with a ones row)
so the stationary matmul operand needs no on-chip transpose for layer 1; the
single mid-network transpose runs on the PE.
"""

import numpy as np

B, W, D, H = 256, 32, 64, 4
N = B * W
NCORES = 8
RPC = N // NCORES          # rows per core = 1024
TILES = RPC // 128         # 8 tiles of 128 rows
EPS = 1e-5
LRELU_SLOPE = 0.01


def _fold_weights(inp):
    f = lambda k: np.asarray(inp[k], np.float64)
    M1 = f("Wl1") @ f("linw1") @ f("w1")
    v1 = (f("bl1") + f("cb1")) @ f("linw1") @ f("w1") + f("b1")
    A2w = f("Wl2") @ f("linw2") @ f("w2")
    M2 = f("gamma")[:, None] * A2w
    v2 = f("beta") @ A2w + (f("bl2") + f("cb2")) @ f("linw2") @ f("w2") + f("b2")
    Cm = np.eye(D) - 1.0 / D
    M2c = Cm @ M2
    m1a = np.concatenate([M1, v1[None, :]], 0).astype(np.float32)  # [65, 64]
    m2a = np.concatenate([M2c, v2[None, :]], 0).astype(np.float32)  # [65, 64]
    return m1a, m2a


def _edges_degenerate(src, dst):
    src = np.asarray(src)
    dst = np.asarray(dst)
    return src.shape == dst.shape and np.array_equal(src, dst) and np.all(
        np.bincount(dst.astype(np.int64), minlength=N)[:N] > 0
    )


def _numpy_fallback(inp):
    # Generic (slow) host implementation, only used if the edge arrays ever
    # stop being fully degenerate.
    x = np.asarray(inp["x"], np.float32).reshape(N, D)
    src = np.asarray(inp["edge_src"]).astype(np.int64)
    dst = np.asarray(inp["edge_dst"]).astype(np.int64)

    def gat(xf, Wl, bl, Wr, br, att, cb, linw):
        xl = (xf @ Wl + bl).reshape(N, H, D)
        xr = (xf @ Wr + br).reshape(N, H, D)
        e = xl[src] + xr[dst]
        e = np.where(e > 0, e, 0.2 * e)
        logits = np.einsum("ehd,hd->eh", e, att)
        m = np.full((N, H), -np.inf, np.float32)
        np.maximum.at(m, dst, logits)
        ex = np.exp(logits - m[dst])
        den = np.zeros((N, H), np.float32)
        np.add.at(den, dst, ex)
        alpha = ex / den[dst]
        out = np.zeros((N, H, D), np.float32)
        np.add.at(out, dst, xl[src] * alpha[:, :, None])
        return (out.reshape(N, H * D) + cb) @ linw

    g = lambda k: np.asarray(inp[k], np.float32)
    lr = lambda t, a: np.where(t > 0, t, a * t)
    out = gat(x, g("Wl1"), g("bl1"), g("Wr1"), g("br1"), g("att1"), g("cb1"), g("linw1"))
    out = lr(out @ g("w1") + g("b1"), 0.01)
    mu = out.mean(-1, keepdims=True)
    var = ((out - mu) ** 2).mean(-1, keepdims=True)
    out = (out - mu) / np.sqrt(var + EPS) * g("gamma") + g("beta")
    out = gat(out, g("Wl2"), g("bl2"), g("Wr2"), g("br2"), g("att2"), g("cb2"), g("linw2"))
    out = lr(out @ g("w2") + g("b2"), 0.01)
    return out.reshape(B, W, D).astype(np.float32)


def build_bass():
    from concourse import bacc, mybir
    import concourse.tile as tile
    from concourse.masks import make_identity

    fp32 = mybir.dt.float32
    Act = mybir.ActivationFunctionType
    Alu = mybir.AluOpType

    nc = bacc.Bacc()
    xat_d = nc.declare_dram_parameter("xat", [D + 1, RPC], fp32, isOutput=False)
    m1_d = nc.declare_dram_parameter("m1a", [D + 1, D], fp32, isOutput=False)
    m2_d = nc.declare_dram_parameter("m2a", [D + 1, D], fp32, isOutput=False)
    y_d = nc.declare_dram_parameter("y", [RPC, D], fp32, isOutput=True)

    with tile.TileContext(nc) as tc:
        with (
            tc.tile_pool(name="const", bufs=1) as cpool,
            tc.tile_pool(name="psum", bufs=1, space="PSUM") as ppool,
            tc.tile_pool(name="work", bufs=3) as wpool,
        ):
            # ---- constants / persistent tiles ----
            ident = cpool.tile([128, 128], fp32, tag="ident")
            make_identity(nc, ident[:])
            xat = cpool.tile([D + 1, RPC], fp32, tag="xat")
            m1 = cpool.tile([D + 1, D], fp32, tag="m1")
            m2 = cpool.tile([D + 1, D], fp32, tag="m2")
            t_all = cpool.tile([128, TILES * D], fp32, tag="t_all")
            s1 = cpool.tile([128, TILES], fp32, tag="s1")
            s2 = cpool.tile([128, TILES], fp32, tag="s2")
            stats = cpool.tile([128, 4 * TILES], fp32, tag="stats")
            epsb = cpool.tile([128, 1], fp32, tag="epsb")
            nc.vector.memset(epsb[:], EPS)
            ones_row = cpool.tile([1, 128], fp32, tag="ones_row")
            nc.vector.memset(ones_row[:], 1.0)
            warm = cpool.tile([1, 1], fp32, tag="warm")
            # persistent PSUM tiles: disjoint column slices per row-tile, so
            # there is no slot recycling and no cross-engine release waits on
            # PE matmuls (HW allows one sync-wait per LDWEIGHTS slot).
            p1big = ppool.tile([128, TILES * D], fp32, tag="p1big")
            p2big = ppool.tile([128, TILES * D], fp32, tag="p2big")
            pTbig = ppool.tile([D, TILES * 128], fp32, tag="pTbig")
            wp = ppool.tile([D, 1], fp32, tag="wp")

            # ACT table warm-up: force the sqrt_and_others set (which also
            # contains leaky_relu/square/copy) to load while input DMA runs.
            nc.vector.memset(warm[:], 1.0)
            nc.scalar.activation(out=warm[:], in_=warm[:], func=Act.Sqrt)

            # ---- weight + input DMA ----
            v2row = cpool.tile([1, D], fp32, tag="v2row")
            nc.sync.dma_start(out=m1[:], in_=m1_d[:])
            nc.sync.dma_start(out=m2[:], in_=m2_d[:])
            nc.sync.dma_start(out=v2row[:], in_=m2_d[D:D + 1, :])
            NCHUNK = 4
            cw = RPC // NCHUNK
            for c in range(NCHUNK):
                nc.sync.dma_start(
                    out=xat[:, c * cw:(c + 1) * cw], in_=xat_d[:, c * cw:(c + 1) * cw]
                )

            # PE pre-consume of each weight DMA (one accumulation group):
            # the PE observes each DMA semaphore here, so the real matmuls
            # below need at most one wait each.
            nc.tensor.matmul(out=wp[:], lhsT=m1[0:D + 1, 0:D], rhs=m1[:, 0:1],
                             start=True, stop=False)
            nc.tensor.matmul(out=wp[:], lhsT=m2[0:D + 1, 0:D], rhs=m2[:, 0:1],
                             start=False, stop=False)
            nc.tensor.matmul(out=wp[:], lhsT=v2row[:], rhs=v2row[:, 0:1],
                             start=False, stop=True)

            # ---- phase A: t = lrelu(x @ M1 + v1), accumulate row stats ----
            for i in range(TILES):
                p1 = p1big[:, i * D:(i + 1) * D]
                nc.tensor.matmul(
                    out=p1,
                    lhsT=xat[:, i * 128:(i + 1) * 128],
                    rhs=m1[:],
                    start=True,
                    stop=True,
                )
                tsl = t_all[:, i * D:(i + 1) * D]
                # leaky_relu(x) = max(0.01*x, x), exact; two ops since only
                # one non-scalar PSUM read is allowed per instruction.
                lp = wpool.tile([128, D], fp32, tag="lp")
                nc.vector.tensor_scalar(
                    out=lp[:], in0=p1, scalar1=LRELU_SLOPE, scalar2=None,
                    op0=Alu.mult,
                )
                nc.vector.scalar_tensor_tensor(
                    out=tsl, in0=lp[:], scalar=1.0, in1=p1,
                    op0=Alu.mult, op1=Alu.max, accum_out=s1[:, i:i + 1],
                )
                sq = wpool.tile([128, D], fp32, tag="sq")
                nc.scalar.activation(
                    out=sq[:], in_=tsl, func=Act.Square, accum_out=s2[:, i:i + 1]
                )

            # ---- phase B: per-row scale a = rsqrt(var + eps), batched ----
            u = stats[:, 0:TILES]
            msq = stats[:, TILES:2 * TILES]
            var = stats[:, 2 * TILES:3 * TILES]
            a_all = stats[:, 3 * TILES:4 * TILES]
            nc.vector.tensor_scalar(
                out=u, in0=s1[:], scalar1=1.0 / D, scalar2=None, op0=Alu.mult
            )
            nc.vector.tensor_tensor(out=msq, in0=u, in1=u, op=Alu.mult)
            nc.vector.scalar_tensor_tensor(
                out=var, in0=s2[:], scalar=1.0 / D, in1=msq,
                op0=Alu.mult, op1=Alu.subtract,
            )
            sd = wpool.tile([128, TILES], fp32, tag="sd")
            nc.scalar.activation(out=sd[:], in_=var, func=Act.Sqrt, bias=epsb[:])
            nc.vector.reciprocal(out=a_all, in_=sd[:])

            # ---- phase C: out = lrelu((a*t) @ M2c + v2) ----
            for i in range(TILES):
                ta = wpool.tile([128, D], fp32, tag="ta")
                nc.vector.tensor_scalar(
                    out=ta[:], in0=t_all[:, i * D:(i + 1) * D],
                    scalar1=a_all[:, i:i + 1], scalar2=None, op0=Alu.mult,
                )
                pT = pTbig[:, i * 128:(i + 1) * 128]
                nc.tensor.transpose(out=pT, in_=ta[:], identity=ident[:])
                taT = wpool.tile([D, 128], fp32, tag="taT")
                nc.vector.tensor_copy(out=taT[:], in_=pT)
                p2 = p2big[:, i * D:(i + 1) * D]
                nc.tensor.matmul(
                    out=p2, lhsT=taT[:], rhs=m2[0:D, :], start=True, stop=False
                )
                # + ones(128) x v2 : bias add via PSUM accumulation
                nc.tensor.matmul(
                    out=p2, lhsT=ones_row[:], rhs=v2row[:],
                    start=False, stop=True,
                )
                lp2 = wpool.tile([128, D], fp32, tag="lp2")
                nc.vector.tensor_scalar(
                    out=lp2[:], in0=p2, scalar1=LRELU_SLOPE, scalar2=None,
                    op0=Alu.mult,
                )
                o = wpool.tile([128, D], fp32, tag="o")
                nc.vector.tensor_tensor(
                    out=o[:], in0=lp2[:], in1=p2, op=Alu.max,
                )
                nc.sync.dma_start(out=y_d[i * 128:(i + 1) * 128, :], in_=o[:])

    return nc


def kernel(**inputs):
    if not _edges_degenerate(inputs["edge_src"], inputs["edge_dst"]):
        return _numpy_fallback(inputs)

    from concourse.bass_utils import run_bass_kernel_spmd

    m1a, m2a = _fold_weights(inputs)
    xf = np.ascontiguousarray(np.asarray(inputs["x"], np.float32).reshape(N, D))
    ones = np.ones((RPC, 1), np.float32)
    in_maps = []
    for c in range(NCORES):
        xs = xf[c * RPC:(c + 1) * RPC]
        xat = np.ascontiguousarray(np.concatenate([xs, ones], 1).T)  # [65, 1024]
        in_maps.append({"xat": xat, "m1a": m1a, "m2a": m2a})

    nc = build_bass()
    if not nc.is_finalized():
        nc.finalize()
    res = run_bass_kernel_spmd(nc, in_maps, list(range(NCORES)))
    global LAST_RESULT
    LAST_RESULT = res
    out = np.concatenate([r["y"] for r in res.results], 0)
    return out.reshape(B, W, D).astype(np.float32)


LAST_RESULT = None


if __name__ == "__main__":
    x = np.random.randn(B, W, D).astype(np.float32)
    print("kernel module ok")



# revision 2
# speedup vs baseline: 1.0147x; 1.0147x over previous
"""Trainium2 Bass kernel v4 for nn_HSR_2_25116968747549 (gnn_message_passing).

Math: degenerate self-edge graph => the network collapses to
    t   = prelu(x @ M1 + v1, 0.01)
    a   = rsqrt(var_row(t) + eps)        (LayerNorm fold; mean removal and
                                          gamma folded into M2c host-side)
    out = prelu(a * (t @ M2c) + v2, 0.01)

Implementation highlights (evolved v1->v4 by trace analysis):
 * bf16 matmuls (fp32 matmuls run LOW_HIGH double-pass: ~4x slower).
 * Phase A flipped: host sends xT [64,1024]; 1 weight load + 2 N=512
   matmuls; ACT hardware Prelu (alpha=0.01, per-partition v1 bias) -> bf16.
 * t and t^2 stacked in one [128,1024] SBUF tile; ONE matmul per 128-row
   tile computes z=t@M2c, s1=sum(t), s2=sum(t^2) (cols 64/65)  -- no
   transposes, no separate stats matmuls, row-major output.
 * Prelu (parametric_relu) shares the ACT table with Sqrt -> no mid-kernel
   ACT table reloads (Lrelu does NOT: it lives in a sqrt-less table).
 * ONE input DMA + ONE weight DMA (v1 bias bitcast-packed into the bf16
   weight tensor): DMA completion semaphores lag ~2us and serialize near
   global queue drain, so fewer DMAs => earlier compute start.
 * PE warm-up dummy matmuls during the DMA window (HAM clock gate: PE runs
   1.2 GHz until ~3.4us of sustained activity, then 2.4 GHz).
 * Per-half stats + wide broadcast finish: stride-0 broadcast APs let one
   tensor_tensor + one scalar_tensor_tensor handle 4 tiles at once.
 * Output halves on two different HWDGE queues (Sync + ACT).
"""

import numpy as np

B, W, D, H = 256, 32, 64, 4
N = B * W
NCORES = 8
RPC = N // NCORES          # rows per core = 1024
TILES = RPC // 128         # 8 tiles of 128 rows
EPS = 1e-5
SLOPE = 0.01
NDUMMY = 4                 # PE warm-up matmuls


def _fold_weights(inp):
    f = lambda k: np.asarray(inp[k], np.float64)
    M1 = f("Wl1") @ f("linw1") @ f("w1")
    v1 = (f("bl1") + f("cb1")) @ f("linw1") @ f("w1") + f("b1")
    A2w = f("Wl2") @ f("linw2") @ f("w2")
    M2 = f("gamma")[:, None] * A2w
    v2 = f("beta") @ A2w + (f("bl2") + f("cb2")) @ f("linw2") @ f("w2") + f("b2")
    Cm = np.eye(D) - 1.0 / D
    M2c = Cm @ M2
    return M1, v1, M2c, v2


def _edges_degenerate(src, dst):
    src = np.asarray(src)
    dst = np.asarray(dst)
    return src.shape == dst.shape and np.array_equal(src, dst) and np.all(
        np.bincount(dst.astype(np.int64), minlength=N)[:N] > 0
    )


def _numpy_fallback(inp):
    x = np.asarray(inp["x"], np.float32).reshape(N, D)
    src = np.asarray(inp["edge_src"]).astype(np.int64)
    dst = np.asarray(inp["edge_dst"]).astype(np.int64)

    def gat(xf, Wl, bl, Wr, br, att, cb, linw):
        xl = (xf @ Wl + bl).reshape(N, H, D)
        xr = (xf @ Wr + br).reshape(N, H, D)
        e = xl[src] + xr[dst]
        e = np.where(e > 0, e, 0.2 * e)
        logits = np.einsum("ehd,hd->eh", e, att)
        m = np.full((N, H), -np.inf, np.float32)
        np.maximum.at(m, dst, logits)
        ex = np.exp(logits - m[dst])
        den = np.zeros((N, H), np.float32)
        np.add.at(den, dst, ex)
        alpha = ex / den[dst]
        out = np.zeros((N, H, D), np.float32)
        np.add.at(out, dst, xl[src] * alpha[:, :, None])
        return (out.reshape(N, H * D) + cb) @ linw

    g = lambda k: np.asarray(inp[k], np.float32)
    lr = lambda t, a: np.where(t > 0, t, a * t)
    out = gat(x, g("Wl1"), g("bl1"), g("Wr1"), g("br1"), g("att1"), g("cb1"), g("linw1"))
    out = lr(out @ g("w1") + g("b1"), 0.01)
    mu = out.mean(-1, keepdims=True)
    var = ((out - mu) ** 2).mean(-1, keepdims=True)
    out = (out - mu) / np.sqrt(var + EPS) * g("gamma") + g("beta")
    out = gat(out, g("Wl2"), g("bl2"), g("Wr2"), g("br2"), g("att2"), g("cb2"), g("linw2"))
    out = lr(out @ g("w2") + g("b2"), 0.01)
    return out.reshape(B, W, D).astype(np.float32)


def build_bass():
    from concourse import bacc, mybir
    import concourse.tile as tile

    fp32 = mybir.dt.float32
    bf16 = mybir.dt.bfloat16
    Act = mybir.ActivationFunctionType
    Alu = mybir.AluOpType

    nc = bacc.Bacc()
    # wt layout (bf16 [128, 196]):
    #   [0:64, 0:64]   = M1                      (phase-A lhsT)
    #   [:, 64:130]    = stacked phase-C rhs:
    #        rows 0:64:   [M2c | ones | 0]
    #        rows 64:128: [ 0  |  0   | ones]
    #   [0, 130:194]   = v2                      (broadcast outer-product row)
    #   [0:64, 194:196] = v1 as fp32 bytes       (bitcast ACT bias column)
    xt_d = nc.declare_dram_parameter("xt", [D, RPC], bf16, isOutput=False)
    wt_d = nc.declare_dram_parameter("wt", [D, 196], bf16, isOutput=False)
    # partition-major output: each SBUF partition writes one contiguous
    # 2KB DRAM row -> 128 big DMA descriptors instead of 1024 small ones
    # (descriptor generation was ~1.4us for the 256B-chunk layout).
    y_d = nc.declare_dram_parameter("y", [128, TILES * D], fp32, isOutput=True)

    with tile.TileContext(nc) as tc:
        with (
            tc.tile_pool(name="const", bufs=1) as cpool,
            tc.tile_pool(name="psum", bufs=1, space="PSUM") as ppool,
        ):
            xt = cpool.tile([D, RPC], bf16, tag="xt")
            wt = cpool.tile([128, 196], bf16, tag="wt")
            tsq = cpool.tile([128, RPC], bf16, tag="tsq")
            onesb = cpool.tile([1, 128], bf16, tag="onesb")
            epsb = cpool.tile([128, 1], fp32, tag="epsb")
            v2b = cpool.tile([128, D], fp32, tag="v2b")
            stats = cpool.tile([128, 5 * TILES], fp32, tag="stats")
            u_sb = cpool.tile([128, TILES * D], fp32, tag="u_sb")
            o_sb = cpool.tile([128, TILES, D], fp32, tag="o_sb")
            warm = cpool.tile([1, 1], fp32, tag="warm")

            # split PSUM tiles: dependency tracking is tensor-granular, so
            # separate tiles per pipeline half keep deps fine-grained
            pA0 = ppool.tile([D, 512], fp32, tag="pA0")            # bank 0
            pA1 = ppool.tile([D, 512], fp32, tag="pA1")            # bank 1
            pC0 = ppool.tile([128, 2, 2, 256], fp32, tag="pC0")    # banks 2-3
            pC1 = ppool.tile([128, 2, 2, 256], fp32, tag="pC1")    # banks 4-5
            pV = ppool.tile([128, D], fp32, tag="pV")              # bank 6
            pCs = (pC0, pC1)
            pAs = (pA0, pA1)

            # constants + ACT table warm-up (Prelu and Sqrt share a table)
            nc.vector.memset(onesb[:], 1.0)
            nc.vector.memset(epsb[:], EPS)
            nc.vector.memset(warm[:], 1.0)
            nc.scalar.activation(out=warm[:], in_=warm[:], func=Act.Prelu,
                                 alpha=SLOPE)

            # input halves on Sync queue; weights on ACT queue -- only rows
            # 0:64 come from DRAM, rows 64:128 of the stacked block are
            # constants built by memset
            nc.vector.memset(wt[D:128, 64:130], 0.0)
            nc.vector.memset(wt[D:128, 129:130], 1.0 / D)
            nc.sync.dma_start(out=xt[:, 0:512], in_=xt_d[:, 0:512])
            nc.sync.dma_start(out=xt[:, 512:1024], in_=xt_d[:, 512:1024])
            nc.scalar.dma_start(out=wt[0:D, :], in_=wt_d[:])


            # v2 broadcast tile: ones(128) (x) v2  ->  [128, 64]
            nc.tensor.matmul(out=pV[:], lhsT=onesb[:],
                             rhs=wt[0:1, 130:194], start=True, stop=True)
            nc.vector.tensor_copy(out=v2b[:], in_=pV[:])

            # phase A: tT = Prelu(M1^T xT + v1)
            wc_ap = wt[0:D, 194:196].bitcast(fp32)
            for h in range(2):
                sl = slice(512 * h, 512 * (h + 1))
                nc.tensor.matmul(
                    out=pAs[h][:], lhsT=wt[0:D, 0:D],
                    rhs=xt[:, sl], start=True, stop=True,
                )
                nc.scalar.activation(
                    out=tsq[0:D, sl], in_=pAs[h][:], func=Act.Prelu,
                    bias=wc_ap, scale=1.0, alpha=SLOPE,
                )
                nc.vector.tensor_tensor(
                    out=tsq[D:128, sl], in0=tsq[0:D, sl],
                    in1=tsq[0:D, sl], op=Alu.mult,
                )

            # phase C: one matmul per tile -> z | s1 | s2
            for i in range(TILES):
                nc.tensor.matmul(
                    out=pCs[i // 4][:, (i % 4) // 2, i % 2, 0:66],
                    lhsT=tsq[:, 128 * i:128 * (i + 1)],
                    rhs=wt[:, 64:130], start=True, stop=True,
                )

            # stats: per-half moments (each waits only its half's matmuls),
            # one shared Sqrt + reciprocal
            u = stats[:, 0:8]
            msq = stats[:, 8:16]
            var = stats[:, 16:24]
            sd = stats[:, 24:32]
            a8 = stats[:, 32:40]
            for h in range(2):
                c4 = slice(4 * h, 4 * (h + 1))
                s1 = pCs[h][:, :, :, 64:65]
                s2 = pCs[h][:, :, :, 65:66]
                nc.vector.tensor_scalar(
                    out=u[:, c4], in0=s1, scalar1=1.0, scalar2=None,
                    op0=Alu.mult)
                nc.vector.tensor_tensor(
                    out=msq[:, c4], in0=u[:, c4], in1=u[:, c4], op=Alu.mult)
                nc.vector.scalar_tensor_tensor(
                    out=var[:, c4], in0=s2, scalar=1.0, in1=msq[:, c4],
                    op0=Alu.mult, op1=Alu.subtract)
            nc.scalar.activation(out=a8[:], in_=var[:],
                                 func=Act.Abs_reciprocal_sqrt, bias=epsb[:])

            # wide broadcast finish per half + output on two queues
            yv = y_d[:]
            for h in range(2):
                c4 = slice(4 * h, 4 * (h + 1))
                usl = slice(256 * h, 256 * (h + 1))
                nc.vector.tensor_tensor(
                    out=u_sb[:, usl], in0=pCs[h][:, :, :, 0:64],
                    in1=a8[:, c4].unsqueeze(2).broadcast_to([128, 4, 64]),
                    op=Alu.mult)
                nc.vector.scalar_tensor_tensor(
                    out=u_sb[:, usl], in0=u_sb[:, usl], scalar=1.0,
                    in1=v2b[:].unsqueeze(1).broadcast_to([128, 4, 64]),
                    op0=Alu.mult, op1=Alu.add)
                nc.scalar.activation(
                    out=o_sb[:, c4, :], in_=u_sb[:, usl],
                    func=Act.Prelu, alpha=SLOPE)
                if h == 0:
                    nc.sync.dma_start(out=yv[:, 0:256], in_=o_sb[:, 0:4, :])
                else:
                    nc.scalar.dma_start(out=yv[:, 256:512], in_=o_sb[:, 4:8, :])

    return nc


def _prep_inputs(inp):
    import ml_dtypes
    M1, v1, M2c, v2 = _fold_weights(inp)
    wt = np.zeros((D, 196), np.float32)
    wt[0:D, 0:64] = M1
    wt[0:D, 64:128] = M2c
    wt[0:D, 128] = 1.0 / D
    wt[0, 130:194] = v2
    wt = wt.astype(ml_dtypes.bfloat16)
    wt[0:D, 194:196] = (
        v1.astype(np.float32).reshape(D, 1).view(ml_dtypes.bfloat16))

    xf = np.asarray(inp["x"], np.float32).reshape(N, D)
    in_maps = []
    for c in range(NCORES):
        xs = xf[c * RPC:(c + 1) * RPC]
        xtc = np.ascontiguousarray(xs.T).astype(ml_dtypes.bfloat16)
        in_maps.append({"xt": xtc, "wt": wt})
    return in_maps


def kernel(**inputs):
    if not _edges_degenerate(inputs["edge_src"], inputs["edge_dst"]):
        return _numpy_fallback(inputs)

    from concourse.bass_utils import run_bass_kernel_spmd

    in_maps = _prep_inputs(inputs)
    nc = build_bass()
    if not nc.is_finalized():
        nc.finalize()
    res = run_bass_kernel_spmd(nc, in_maps, list(range(NCORES)))
    global LAST_RESULT
    LAST_RESULT = res
    outs = []
    for r in res.results:
        yc = r["y"].reshape(128, TILES, D).transpose(1, 0, 2).reshape(RPC, D)
        outs.append(yc)
    out = np.concatenate(outs, 0)
    return out.reshape(B, W, D).astype(np.float32)


LAST_RESULT = None


if __name__ == "__main__":
    print("kernel v4 module ok")
